# revision 32
# baseline (speedup 1.0000x reference)
"""Single-head causal self-attention on 8 TRN2 NeuronCores.

Problem (hardcoded): x [8, 2048, 1024] f32, Wq/Wk/Wv [1024, 1024] f32.
  Q = x@Wq; K = x@Wk; V = x@Wv
  A = (Q K^T) / sqrt(1024), causal; P = softmax(A); out = P V   -> [8, 2048, 1024] f32

Sharding: batch-parallel - core b computes batch element b, no collectives.

Mixed-precision fp8/fp16 design (rel-err budget 2e-2, fp16 baseline 5.9e-4):
  - The first q-chunk (rows 0..255) has a concentrated softmax (row q
    averages over <=q+1 keys), which amplifies input quantization noise
    into the output; it runs fully in fp16 (projections + attention).
  - Rows 256.. have diffuse softmax (sqrt(sum P^2) <= ~0.2), so fp8e4
    noise attenuates to <1e-2 of output absmax. V = xWv, S^T, and P@V
    run in fp8e4 with DoubleRow perf mode (2 contraction elements / PE
    cycle). T = x G (G = Wq Wk^T) is STORED fp8; numpy-simulating the
    full quant pipeline showed its COMPUTE precision must be fp16 for
    q-cols < 768 (t8 col q only feeds row q's scores; rows 256..767
    still have concentrated-enough softmax that the extra x8-quant
    noise of fp8-computed T doubles their error) while cols 768+
    tolerate the cheaper fp8 DoubleRow compute at zero error change.
  - WqT is host-scaled x16, so G/T/scores carry a uniform x16 that the
    1/512 exp scale removes; G's sigma=1/32 entries then clear the e4m3
    subnormal floor when T is cast. Wv8 is likewise host-scaled x16,
    cancelled by a 16-valued ones vector in the row-sum matmul.
  - Causal masking: a [128,128] identity matmul adds a -1600/-25600 bias
    tile into the diagonal score PSUM; exp(bias*scale) flushes masked
    entries to exact 0 in fp8/fp16. No gpsimd in the attention loop.
  - exp uses bias -3 (E = e^(s-3)) so E stays below e4m3's 240->Inf
    cliff even for 7-sigma scores; the common factor cancels in r.
"""
import numpy as np

import concourse.bacc as bacc
import concourse.bass as bass
import concourse.mybir as mybir
import concourse.tile as tile

F32 = mybir.dt.float32
F16 = mybir.dt.float16
F8 = mybir.dt.float8e4
DR = mybir.MatmulPerfMode.DoubleRow

B = 8
S = 2048
D = 1024
P = 128
ND = D // P          # 8 d-tiles (contraction tiles)
NS = S // P          # 16 s-tiles
QC = 256             # q-chunk for attention
NQC = S // QC        # 8 chunks
INVS = 1.0 / (32.0 * 16.0)  # exp scale: 1/sqrt(d_model) / (x16 of WqT)
EXPB = -3.0               # exp bias: E = e^(s-3), cancels in normalize
VS = 16.0                 # host scale on Wv8, cancelled via ones8=16
TB = 768                  # t8 cols < TB are computed in fp16


def build():
    nc = bacc.Bacc(None, target_bir_lowering=False)

    xt8_d = nc.dram_tensor("xt8", [D, S], F8, kind="ExternalInput")
    xt16_d = nc.dram_tensor("xt16", [D, TB], F16, kind="ExternalInput")
    wqkt_d = nc.dram_tensor("WqkT", [2, D, D], F16, kind="ExternalInput")
    wv16_d = nc.dram_tensor("Wv16", [D, D], F16, kind="ExternalInput")
    wv8_d = nc.dram_tensor("Wv8", [D, D], F8, kind="ExternalInput")
    id_d = nc.dram_tensor("id16", [P, P], F16, kind="ExternalInput")
    # mask bias rows per diagonal tile position (scores carry x16)
    mb_d = nc.dram_tensor("maskb", [P, 2, QC], F16, kind="ExternalInput")
    ones8_d = nc.dram_tensor("ones8", [P, 2, 1], F8, kind="ExternalInput")
    out_d = nc.dram_tensor("out", [S, D], F32, kind="ExternalOutput")

    with tile.TileContext(nc) as tc:
        with (
            tc.tile_pool(name="consts", bufs=1) as consts,
            tc.tile_pool(name="big", bufs=1) as big,
        ):
            ones16 = consts.tile([P, 1], F16)
            nc.gpsimd.memset(ones16[:], 1.0)
            expb = consts.tile([P, 1], F32)
            nc.gpsimd.memset(expb[:], EXPB)

            xt16 = big.tile([P, ND, TB], F16)   # x^T cols 0:TB, fp16
            x8 = big.tile([P, ND, S], F8)       # x^T full, fp8
            tt16 = big.tile([P, ND, QC], F16)   # 16*T^T q-cols 0:256, fp16
            t8 = big.tile([P, ND, S], F8)       # 16*T^T, q-cols 256:, fp8
            v16 = big.tile([P, 2, D], F16)      # V rows 0:256, fp16
            v8 = big.tile([P, NS, D], F8)       # 16*V, fp8
            g16 = big.tile([P, ND, D], F16)     # 16*G, G = Wq Wk^T
            g8 = big.tile([P, ND, D], F8)       # 16*G, fp8
            wv16s = big.tile([P, ND, D], F16)
            wv8s = big.tile([P, ND, D], F8)
            id16 = consts.tile([P, P], F16)
            maskb = consts.tile([P, 2, QC], F16)
            ones8 = consts.tile([P, 2, 1], F8)
            wqk_t = [big.tile([P, 2, ND, 256], F16, name=f"wqk{c}")
                     for c in range(4)]

            # ---- input DMAs, one ring, FIFO in consumption order ----
            wqk_src = wqkt_d[:, :, :].rearrange("w (a p) n -> p w a n", p=P)
            for c in range(4):
                sl = slice(256 * c, 256 * (c + 1))
                nc.sync.dma_start(wqk_t[c][:], wqk_src[:, :, :, sl])
            nc.sync.dma_start(id16[:], id_d[:, :])
            nc.sync.dma_start(maskb[:], mb_d[:, :, :])
            nc.sync.dma_start(ones8[:], ones8_d[:, :, :])
            nc.sync.dma_start(
                xt16[:], xt16_d[:, :].rearrange("(a p) s -> p a s", p=P))
            nc.sync.dma_start(
                x8[:], xt8_d[:, :].rearrange("(a p) s -> p a s", p=P))
            nc.sync.dma_start(
                wv16s[:], wv16_d[:, :].rearrange("(a p) n -> p a n", p=P))
            nc.sync.dma_start(
                wv8s[:], wv8_d[:, :].rearrange("(a p) n -> p a n", p=P))

            with (
                tc.tile_pool(name="projp", bufs=3, space="PSUM") as projp,
                tc.tile_pool(name="warmp", bufs=1, space="PSUM") as warmp,
            ):
                ncopy = 0

                def psum_out(dst_ap, ps, k):
                    if k % 2 == 0:
                        nc.vector.tensor_copy(dst_ap, ps[:])
                    else:
                        nc.scalar.copy(dst_ap, ps[:])

                # PE warmup sized to end as the first W^T chunks land.
                dum = consts.tile([P, 512], F16)
                nc.gpsimd.memset(dum[:], 0.0)
                wtile = warmp.tile([P, 512], F32)
                for _ in range(12):
                    nc.tensor.matmul(wtile[:], dum[:, 0:P], dum[:],
                                     start=True, stop=True)

                # G = Wq Wk^T (fp16): out[d 128 (m), e 256 (c)], group
                # order so level L only needs the first L+1 W^T chunks.
                gpairs = sorted(
                    ((m, c) for m in range(ND) for c in range(D // 256)),
                    key=lambda mc: (max(mc[0] // 2, mc[1]), mc[1], mc[0]))
                for m, c in gpairs:
                    ps = projp.tile([P, 256], F32, name="gps")
                    for a in range(ND):
                        nc.tensor.matmul(
                            ps[:],
                            wqk_t[m // 2][:, 0, a,
                                          P * (m % 2):P * (m % 2 + 1)],
                            wqk_t[c][:, 1, a, :],
                            start=(a == 0), stop=(a == ND - 1))
                    # two copies: fp16 master (DVE) + fp8 (ACT)
                    sl = slice(256 * c, 256 * (c + 1))
                    nc.vector.tensor_copy(g16[:, m, sl], ps[:])
                    nc.scalar.copy(g8[:, m, sl], ps[:])

                # 16*T^T q-cols 0:256 (fp16): out[e 128 (m), q 256]
                for m in range(ND):
                    ps = projp.tile([P, QC], F32)
                    for a in range(ND):
                        nc.tensor.matmul(
                            ps[:],
                            g16[:, a, P * m:P * (m + 1)],
                            xt16[:, a, 0:QC],
                            start=(a == 0), stop=(a == ND - 1))
                    psum_out(tt16[:, m, :], ps, ncopy)
                    ncopy += 1
                # V rows 0:256 (fp16): out[s 128 (i), dv 512 (h)]
                for i in range(2):
                    for h in range(D // 512):
                        ps = projp.tile([P, 512], F32)
                        for a in range(ND):
                            nc.tensor.matmul(
                                ps[:],
                                xt16[:, a, P * i:P * (i + 1)],
                                wv16s[:, a, 512 * h:512 * (h + 1)],
                                start=(a == 0), stop=(a == ND - 1))
                        psum_out(v16[:, i, 512 * h:512 * (h + 1)], ps, ncopy)
                        ncopy += 1

                # 16*T^T q-cols 256:TB: fp16 compute (precision), fp8 store
                for qlo, qw in [(256, 512)]:
                    for m in range(ND):
                        ps = projp.tile([P, 512], F32)
                        for a in range(ND):
                            nc.tensor.matmul(
                                ps[:, 0:qw],
                                g16[:, a, P * m:P * (m + 1)],
                                xt16[:, a, qlo:qlo + qw],
                                start=(a == 0), stop=(a == ND - 1))
                        psum_out(t8[:, m, qlo:qlo + qw], ps[:, 0:qw], ncopy)
                        ncopy += 1
                # 16*T^T q-cols TB:2048: fp8 DoubleRow compute
                for qlo, qw in [(768, 512), (1280, 512), (1792, 256)]:
                    for m in range(ND):
                        ps = projp.tile([P, 512], F32)
                        for a in range(ND // 2):
                            nc.tensor.matmul(
                                ps[:, 0:qw],
                                g8[:, 2 * a:2 * a + 2, P * m:P * (m + 1)],
                                x8[:, 2 * a:2 * a + 2, qlo:qlo + qw],
                                start=(a == 0), stop=(a == ND // 2 - 1),
                                perf_mode=DR)
                        psum_out(t8[:, m, qlo:qlo + qw], ps[:, 0:qw], ncopy)
                        ncopy += 1
                # 16*V (fp8 DoubleRow): out[s 128 (i), dv 512 (h)]
                for i in range(NS):
                    for h in range(D // 512):
                        ps = projp.tile([P, 512], F32)
                        for a in range(ND // 2):
                            nc.tensor.matmul(
                                ps[:],
                                x8[:, 2 * a:2 * a + 2, P * i:P * (i + 1)],
                                wv8s[:, 2 * a:2 * a + 2,
                                     512 * h:512 * (h + 1)],
                                start=(a == 0), stop=(a == ND // 2 - 1),
                                perf_mode=DR)
                        psum_out(v8[:, i, 512 * h:512 * (h + 1)], ps, ncopy)
                        ncopy += 1

            # ---- attention over q-chunks of 256, k-tile PAIRS of 256 ----
            # Pass (j, c) computes S^T + exp for k-tiles 2c, 2c+1 into a
            # paired E tile [P, 2, QC]; PV/r run as deferred PIPE passes
            # (DoubleRow over the k-pair for j>=1, fp16 per-tile for j=0).
            # PV is split into two dv-halves on separate PSUM bank pairs
            # (opa: dv 0:512, opb: dv 512:1024): half-A's normalize+DMA
            # runs while half-B accumulates, so the next chunk's PV never
            # waits on bank recycling (was ~1.2us x 7 chunk boundaries).
            PIPE = 2
            with (
                tc.tile_pool(name="stp", bufs=2, space="PSUM") as stp,
                tc.tile_pool(name="opa", bufs=1, space="PSUM") as opa,
                tc.tile_pool(name="opb", bufs=1, space="PSUM") as opb,
                tc.tile_pool(name="rp", bufs=1, space="PSUM") as rp,
                tc.tile_pool(name="ep", bufs=NQC + PIPE + 2) as ep,
                tc.tile_pool(name="osbp", bufs=4) as osbp,
                tc.tile_pool(name="rrp", bufs=2) as rrp,
            ):
                o_ps = {}
                r_ps = {}
                chunk_ets = {}
                rrecs = {}

                def emit_pair_scores(j, c):
                    # Both k-tiles of the pair accumulate into ONE PSUM
                    # bank as a single group (start only on the very
                    # first matmul: untouched bytes stay pending-zero, so
                    # each half's first write lands on zero), closed by a
                    # single [P, 2, QC] exp -> halved ACT instruction
                    # count and pair-granular st freeing.
                    fp8 = j > 0
                    et = ep.tile([P, 2, QC], F8 if fp8 else F16, name="et")
                    st = stp.tile([P, 2, QC], F32, name="st")
                    n_mm = ND // 2 if fp8 else ND
                    for tpar in range(2):
                        t = 2 * c + tpar
                        diag = (t - 2 * j) >= 0  # diagonal-block k-tile
                        for a in range(n_mm):
                            first = (tpar == 0 and a == 0)
                            last = (tpar == 1 and a == n_mm - 1
                                    and not diag)
                            if fp8:
                                nc.tensor.matmul(
                                    st[:, tpar, :],
                                    x8[:, 2 * a:2 * a + 2, P * t:P * (t + 1)],
                                    t8[:, 2 * a:2 * a + 2,
                                       QC * j:QC * (j + 1)],
                                    start=first, stop=last, perf_mode=DR)
                            else:
                                nc.tensor.matmul(
                                    st[:, tpar, :],
                                    xt16[:, a, P * t:P * (t + 1)],
                                    tt16[:, a, :],
                                    start=first, stop=last)
                        if diag:
                            # add -25600 masked-position bias rows
                            nc.tensor.matmul(
                                st[:, tpar, :], id16[:],
                                maskb[:, t - 2 * j, :],
                                start=False, stop=(tpar == 1))
                    nc.scalar.activation(
                        et[:, :, :], st[:],
                        mybir.ActivationFunctionType.Exp,
                        scale=INVS, bias=expb[:])
                    return et

                def pv_block(j, c, et, u, dst, cols, rhs16, rhs8,
                             with_r):
                    # one (pair, u) PV contribution into dst[:, cols]
                    fp8 = j > 0
                    last = (c == j)
                    if fp8:
                        lhsT = et[:, :, P * u:P * (u + 1)]
                        if with_r:
                            nc.tensor.matmul(
                                r_ps[j][u][:], lhsT, ones8[:],
                                start=(c == 0), stop=last, perf_mode=DR)
                        nc.tensor.matmul(
                            dst[:], lhsT, rhs8[:, 2 * c:2 * c + 2, cols],
                            start=(c == 0), stop=last, perf_mode=DR)
                    else:
                        # j=0, single pair: tile 1 is all-zero for u=0
                        tp = [0] if u == 0 else [0, 1]
                        for ti, tpar in enumerate(tp):
                            lhsT = et[:, tpar, P * u:P * (u + 1)]
                            st_, sp_ = (ti == 0), (ti == len(tp) - 1)
                            if with_r:
                                nc.tensor.matmul(
                                    r_ps[j][u][:], lhsT, ones16[:],
                                    start=st_, stop=sp_)
                            nc.tensor.matmul(
                                dst[:], lhsT, rhs16[:, tpar, cols],
                                start=st_, stop=sp_)

                def emit_norm_half(j, o_half, cols):
                    # u=0 on DVE, u=1 on ACT: halves run in parallel
                    rrec = rrecs[j]
                    for u in range(2):
                        osb = osbp.tile([P, 512], F32, name="osb")
                        qt = 2 * j + u
                        if u == 0:
                            nc.vector.tensor_scalar_mul(
                                osb[:], o_half[u][:], rrec[:, u:u + 1])
                        else:
                            nc.scalar.activation(
                                osb[:], o_half[u][:],
                                mybir.ActivationFunctionType.Copy,
                                scale=rrec[:, u:u + 1])
                        nc.sync.dma_start(
                            out_d[P * qt:P * (qt + 1), cols], osb[:])

                def emit_pv(j, c, et):
                    if c == 0:
                        o_ps[j] = [opa.tile([P, 512], F32, name=f"oA{u}")
                                   for u in range(2)]
                        r_ps[j] = [rp.tile([P, 1], F32, name=f"r_ps{u}")
                                   for u in range(2)]
                    for u in range(2):
                        pv_block(j, c, et, u, o_ps[j][u], slice(0, 512),
                                 v16, v8, with_r=True)
                    if c == j:
                        # h0 done: normalize+DMA it while h1 accumulates
                        rrec = rrp.tile([P, 2], F32, name="rrec")
                        for u in range(2):
                            nc.vector.reciprocal(rrec[:, u:u + 1],
                                                 r_ps[j][u][:])
                        rrecs[j] = rrec
                        emit_norm_half(j, o_ps[j], slice(0, 512))
                        oB = [opb.tile([P, 512], F32, name=f"oB{u}")
                              for u in range(2)]
                        for cc, et2 in enumerate(chunk_ets[j]):
                            for u in range(2):
                                pv_block(j, cc, et2, u, oB[u],
                                         slice(512, D), v16, v8,
                                         with_r=False)
                        emit_norm_half(j, oB, slice(512, D))
                        del o_ps[j], r_ps[j], chunk_ets[j], rrecs[j]

                passes = [(j, c) for j in range(NQC) for c in range(j + 1)]
                pending = []
                for (j, c) in passes:
                    et = emit_pair_scores(j, c)
                    chunk_ets.setdefault(j, []).append(et)
                    pending.append((j, c, et))
                    if len(pending) > PIPE:
                        emit_pv(*pending.pop(0))
                for args in pending:
                    emit_pv(*args)

    nc.finalize()
    return nc


_NC = None


def _get_nc():
    global _NC
    if _NC is None:
        _NC = build()
    return _NC


def prep_inputs(x, Wq, Wk, Wv):
    """Host-side marshaling: shard batch, transpose + cast, constants."""
    import ml_dtypes
    F8NP = ml_dtypes.float8_e4m3

    WqkT16 = np.ascontiguousarray(
        np.stack([np.asarray(Wq).T * VS, np.asarray(Wk).T]),
        dtype=np.float16)
    Wv16 = np.ascontiguousarray(Wv, dtype=np.float16)
    Wv8 = np.ascontiguousarray(np.asarray(Wv) * VS).astype(F8NP)
    id16 = np.eye(P, dtype=np.float16)
    r = np.arange(P)[:, None]
    col = np.arange(QC)[None, :]
    maskb = np.zeros((P, 2, QC), dtype=np.float16)
    maskb[:, 0, :] = np.where(col >= r, 0.0, -25600.0)
    maskb[:, 1, :] = np.where(col >= r + P, 0.0, -25600.0)
    ones8 = np.full((P, 2, 1), VS).astype(F8NP)

    out = []
    for b in range(B):
        xt = np.ascontiguousarray(np.asarray(x[b]).T)
        out.append({
            "xt8": xt.astype(F8NP),
            "xt16": np.ascontiguousarray(xt[:, 0:TB]).astype(np.float16),
            "WqkT": WqkT16, "Wv16": Wv16, "Wv8": Wv8,
            "id16": id16, "maskb": maskb, "ones8": ones8,
        })
    return out


def run(x, Wq, Wk, Wv, **spmd_kwargs):
    from concourse.bass_utils import run_bass_kernel_spmd

    nc = _get_nc()
    in_maps = prep_inputs(x, Wq, Wk, Wv)
    res = run_bass_kernel_spmd(nc, in_maps, core_ids=list(range(B)),
                               **spmd_kwargs)
    out = np.stack([res.results[b]["out"] for b in range(B)], axis=0)
    return out, res


def kernel(x, Wq, Wk, Wv):
    out, _ = run(x, Wq, Wk, Wv)
    return out


# revision 34
# speedup vs baseline: 1.1902x; 1.1902x over previous
"""Single-head causal self-attention on 8 TRN2 NeuronCores.

Problem (hardcoded): x [8, 2048, 1024] f32, Wq/Wk/Wv [1024, 1024] f32.
  Q = x@Wq; K = x@Wk; V = x@Wv
  A = (Q K^T) / sqrt(1024), causal; P = softmax(A); out = P V   -> [8, 2048, 1024] f32

Sharding: batch-parallel - core b computes batch element b, no collectives.

Mixed-precision fp8/fp16 design (rel-err budget 2e-2, fp16 baseline 5.9e-4):
  - The first q-chunk (rows 0..255) has a concentrated softmax (row q
    averages over <=q+1 keys), which amplifies input quantization noise
    into the output; it runs fully in fp16 (projections + attention).
  - Rows 256.. have diffuse softmax (sqrt(sum P^2) <= ~0.2), so fp8e4
    noise attenuates to <1e-2 of output absmax. V = xWv, S^T, and P@V
    run in fp8e4 with DoubleRow perf mode (2 contraction elements / PE
    cycle). T = x G (G = Wq Wk^T) is STORED fp8; numpy-simulating the
    full quant pipeline showed its COMPUTE precision must be fp16 for
    q-cols < 768 (t8 col q only feeds row q's scores; rows 256..767
    still have concentrated-enough softmax that the extra x8-quant
    noise of fp8-computed T doubles their error) while cols 768+
    tolerate the cheaper fp8 DoubleRow compute at zero error change.
  - WqT is host-scaled x16, so G/T/scores carry a uniform x16 that the
    1/512 exp scale removes; G's sigma=1/32 entries then clear the e4m3
    subnormal floor when T is cast. Wv8 is likewise host-scaled x16,
    cancelled by a 16-valued ones vector in the row-sum matmul.
  - Causal masking: a [128,128] identity matmul adds a -1600/-25600 bias
    tile into the diagonal score PSUM; exp(bias*scale) flushes masked
    entries to exact 0 in fp8/fp16. No gpsimd in the attention loop.
  - exp uses bias -3 (E = e^(s-3)) so E stays below e4m3's 240->Inf
    cliff even for 7-sigma scores; the common factor cancels in r.
"""
import numpy as np

import concourse.bacc as bacc
import concourse.bass as bass
import concourse.mybir as mybir
import concourse.tile as tile

F32 = mybir.dt.float32
F16 = mybir.dt.float16
F8 = mybir.dt.float8e4
DR = mybir.MatmulPerfMode.DoubleRow

B = 8
S = 2048
D = 1024
P = 128
ND = D // P          # 8 d-tiles (contraction tiles)
NS = S // P          # 16 s-tiles
QC = 256             # q-chunk for attention
NQC = S // QC        # 8 chunks
INVS = 1.0 / (32.0 * 16.0)  # exp scale: 1/sqrt(d_model) / (x16 of WqT)
EXPB = -3.0               # exp bias: E = e^(s-3), cancels in normalize
VS = 16.0                 # host scale on Wv8, cancelled via ones8=16
TB = 768                  # t8 cols < TB are computed in fp16


def build():
    nc = bacc.Bacc(None, target_bir_lowering=False)

    xt8_d = nc.dram_tensor("xt8", [D, S], F8, kind="ExternalInput")
    xt16_d = nc.dram_tensor("xt16", [D, TB], F16, kind="ExternalInput")
    wqkt_d = nc.dram_tensor("WqkT", [2, D, D], F16, kind="ExternalInput")
    wv16_d = nc.dram_tensor("Wv16", [D, D], F16, kind="ExternalInput")
    wv8_d = nc.dram_tensor("Wv8", [D, D], F8, kind="ExternalInput")
    id_d = nc.dram_tensor("id16", [P, P], F16, kind="ExternalInput")
    # mask bias rows per diagonal tile position (scores carry x16)
    mb_d = nc.dram_tensor("maskb", [P, 2, QC], F16, kind="ExternalInput")
    ones8_d = nc.dram_tensor("ones8", [P, 2, 1], F8, kind="ExternalInput")
    out_d = nc.dram_tensor("out", [S, D], F32, kind="ExternalOutput")

    with tile.TileContext(nc) as tc:
        with (
            tc.tile_pool(name="consts", bufs=1) as consts,
            tc.tile_pool(name="big", bufs=1) as big,
        ):
            ones16 = consts.tile([P, 1], F16)
            nc.gpsimd.memset(ones16[:], 1.0)
            expb = consts.tile([P, 1], F32)
            nc.gpsimd.memset(expb[:], EXPB)

            xt16 = big.tile([P, ND, TB], F16)   # x^T cols 0:TB, fp16
            x8 = big.tile([P, ND, S], F8)       # x^T full, fp8
            tt16 = big.tile([P, ND, QC], F16)   # 16*T^T q-cols 0:256, fp16
            t8 = big.tile([P, ND, S], F8)       # 16*T^T, q-cols 256:, fp8
            v16 = big.tile([P, 2, D], F16)      # V rows 0:256, fp16
            v8 = big.tile([P, NS, D], F8)       # 16*V, fp8
            g16 = big.tile([P, ND, D], F16)     # 16*G, G = Wq Wk^T
            g8 = big.tile([P, ND, D], F8)       # 16*G, fp8
            wv16s = big.tile([P, ND, D], F16)
            wv8s = big.tile([P, ND, D], F8)
            id16 = consts.tile([P, P], F16)
            maskb = consts.tile([P, 2, QC], F16)
            ones8 = consts.tile([P, 2, 1], F8)
            wqk_t = [big.tile([P, 2, ND, 256], F16, name=f"wqk{c}")
                     for c in range(4)]

            # ---- input DMAs, one ring, FIFO in consumption order ----
            wqk_src = wqkt_d[:, :, :].rearrange("w (a p) n -> p w a n", p=P)
            for c in range(4):
                sl = slice(256 * c, 256 * (c + 1))
                nc.sync.dma_start(wqk_t[c][:], wqk_src[:, :, :, sl])
            nc.sync.dma_start(id16[:], id_d[:, :])
            nc.sync.dma_start(maskb[:], mb_d[:, :, :])
            nc.sync.dma_start(ones8[:], ones8_d[:, :, :])
            nc.sync.dma_start(
                xt16[:], xt16_d[:, :].rearrange("(a p) s -> p a s", p=P))
            nc.sync.dma_start(
                x8[:], xt8_d[:, :].rearrange("(a p) s -> p a s", p=P))
            nc.sync.dma_start(
                wv16s[:], wv16_d[:, :].rearrange("(a p) n -> p a n", p=P))
            nc.sync.dma_start(
                wv8s[:], wv8_d[:, :].rearrange("(a p) n -> p a n", p=P))

            with (
                tc.tile_pool(name="projp", bufs=3, space="PSUM") as projp,
                tc.tile_pool(name="warmp", bufs=1, space="PSUM") as warmp,
            ):
                ncopy = 0

                def psum_out(dst_ap, ps, k):
                    if k % 2 == 0:
                        nc.vector.tensor_copy(dst_ap, ps[:])
                    else:
                        nc.scalar.copy(dst_ap, ps[:])

                # PE warmup sized to end as the first W^T chunks land.
                dum = consts.tile([P, 512], F16)
                nc.gpsimd.memset(dum[:], 0.0)
                wtile = warmp.tile([P, 512], F32)
                for _ in range(4):
                    nc.tensor.matmul(wtile[:], dum[:, 0:P], dum[:],
                                     start=True, stop=True)

                # G = Wq Wk^T (fp16): out[d 128 (m), e 256 (c)], group
                # order so level L only needs the first L+1 W^T chunks.
                gpairs = sorted(
                    ((m, c) for m in range(ND) for c in range(D // 256)),
                    key=lambda mc: (max(mc[0] // 2, mc[1]), mc[1], mc[0]))
                for m, c in gpairs:
                    ps = projp.tile([P, 256], F32, name="gps")
                    for a in range(ND):
                        nc.tensor.matmul(
                            ps[:],
                            wqk_t[m // 2][:, 0, a,
                                          P * (m % 2):P * (m % 2 + 1)],
                            wqk_t[c][:, 1, a, :],
                            start=(a == 0), stop=(a == ND - 1))
                    # two copies: fp16 master (DVE) + fp8 (ACT)
                    sl = slice(256 * c, 256 * (c + 1))
                    nc.vector.tensor_copy(g16[:, m, sl], ps[:])
                    nc.scalar.copy(g8[:, m, sl], ps[:])

                # 16*T^T q-cols 0:256 (fp16): out[e 128 (m), q 256]
                for m in range(ND):
                    ps = projp.tile([P, QC], F32)
                    for a in range(ND):
                        nc.tensor.matmul(
                            ps[:],
                            g16[:, a, P * m:P * (m + 1)],
                            xt16[:, a, 0:QC],
                            start=(a == 0), stop=(a == ND - 1))
                    psum_out(tt16[:, m, :], ps, ncopy)
                    ncopy += 1
                # V rows 0:256 (fp16): out[s 128 (i), dv 512 (h)]
                for i in range(2):
                    for h in range(D // 512):
                        ps = projp.tile([P, 512], F32)
                        for a in range(ND):
                            nc.tensor.matmul(
                                ps[:],
                                xt16[:, a, P * i:P * (i + 1)],
                                wv16s[:, a, 512 * h:512 * (h + 1)],
                                start=(a == 0), stop=(a == ND - 1))
                        psum_out(v16[:, i, 512 * h:512 * (h + 1)], ps, ncopy)
                        ncopy += 1

                # 16*T^T q-cols 256:TB: fp16 compute (precision), fp8 store
                for qlo, qw in [(256, 512)]:
                    for m in range(ND):
                        ps = projp.tile([P, 512], F32)
                        for a in range(ND):
                            nc.tensor.matmul(
                                ps[:, 0:qw],
                                g16[:, a, P * m:P * (m + 1)],
                                xt16[:, a, qlo:qlo + qw],
                                start=(a == 0), stop=(a == ND - 1))
                        psum_out(t8[:, m, qlo:qlo + qw], ps[:, 0:qw], ncopy)
                        ncopy += 1
                # 16*T^T q-cols TB:2048: fp8 DoubleRow compute
                for qlo, qw in [(768, 512), (1280, 512), (1792, 256)]:
                    for m in range(ND):
                        ps = projp.tile([P, 512], F32)
                        for a in range(ND // 2):
                            nc.tensor.matmul(
                                ps[:, 0:qw],
                                g8[:, 2 * a:2 * a + 2, P * m:P * (m + 1)],
                                x8[:, 2 * a:2 * a + 2, qlo:qlo + qw],
                                start=(a == 0), stop=(a == ND // 2 - 1),
                                perf_mode=DR)
                        psum_out(t8[:, m, qlo:qlo + qw], ps[:, 0:qw], ncopy)
                        ncopy += 1
                # 16*V (fp8 DoubleRow): out[s 128 (i), dv 512 (h)]
                for i in range(NS):
                    for h in range(D // 512):
                        ps = projp.tile([P, 512], F32)
                        for a in range(ND // 2):
                            nc.tensor.matmul(
                                ps[:],
                                x8[:, 2 * a:2 * a + 2, P * i:P * (i + 1)],
                                wv8s[:, 2 * a:2 * a + 2,
                                     512 * h:512 * (h + 1)],
                                start=(a == 0), stop=(a == ND // 2 - 1),
                                perf_mode=DR)
                        psum_out(v8[:, i, 512 * h:512 * (h + 1)], ps, ncopy)
                        ncopy += 1

            # ---- attention over q-chunks of 256, k-tile PAIRS of 256 ----
            # Pass (j, c) computes S^T + exp for k-tiles 2c, 2c+1 into a
            # paired E tile [P, 2, QC]; PV/r run as deferred PIPE passes
            # (DoubleRow over the k-pair for j>=1, fp16 per-tile for j=0).
            # PV is split into two dv-halves on separate PSUM bank pairs
            # (opa: dv 0:512, opb: dv 512:1024): half-A's normalize+DMA
            # runs while half-B accumulates, so the next chunk's PV never
            # waits on bank recycling (was ~1.2us x 7 chunk boundaries).
            PIPE = 2
            with (
                tc.tile_pool(name="stp", bufs=2, space="PSUM") as stp,
                tc.tile_pool(name="opa", bufs=1, space="PSUM") as opa,
                tc.tile_pool(name="opb", bufs=1, space="PSUM") as opb,
                tc.tile_pool(name="rp", bufs=1, space="PSUM") as rp,
                tc.tile_pool(name="ep", bufs=NQC + PIPE + 2) as ep,
                tc.tile_pool(name="osbp", bufs=4) as osbp,
                tc.tile_pool(name="rrp", bufs=2) as rrp,
            ):
                o_ps = {}
                r_ps = {}
                chunk_ets = {}
                rrecs = {}

                def emit_pair_scores(j, c):
                    # Both k-tiles of the pair accumulate into ONE PSUM
                    # bank as a single group (start only on the very
                    # first matmul: untouched bytes stay pending-zero, so
                    # each half's first write lands on zero), closed by a
                    # single [P, 2, QC] exp -> halved ACT instruction
                    # count and pair-granular st freeing.
                    fp8 = j > 0
                    et = ep.tile([P, 2, QC], F8 if fp8 else F16, name="et")
                    st = stp.tile([P, 2, QC], F32, name="st")
                    n_mm = ND // 2 if fp8 else ND
                    for tpar in range(2):
                        t = 2 * c + tpar
                        tl = t - 2 * j
                        diag = tl >= 0  # diagonal-block k-tile
                        # tl=1 tile: q-cols 0:128 are fully masked; skip
                        # computing them (the id/maskb matmul writes the
                        # -25600 rows onto those pending-zero bytes, so
                        # exp still flushes them to exact 0)
                        qs = slice(P, QC) if tl == 1 else slice(0, QC)
                        for a in range(n_mm):
                            first = (tpar == 0 and a == 0)
                            last = (tpar == 1 and a == n_mm - 1
                                    and not diag)
                            if fp8:
                                nc.tensor.matmul(
                                    st[:, tpar, qs],
                                    x8[:, 2 * a:2 * a + 2, P * t:P * (t + 1)],
                                    t8[:, 2 * a:2 * a + 2,
                                       QC * j + qs.start:QC * j + qs.stop],
                                    start=first, stop=last, perf_mode=DR)
                            else:
                                nc.tensor.matmul(
                                    st[:, tpar, qs],
                                    xt16[:, a, P * t:P * (t + 1)],
                                    tt16[:, a, qs],
                                    start=first, stop=last)
                        if diag:
                            # add -25600 masked-position bias rows
                            nc.tensor.matmul(
                                st[:, tpar, :], id16[:],
                                maskb[:, tl, :],
                                start=False, stop=(tpar == 1))
                    nc.scalar.activation(
                        et[:, :, :], st[:],
                        mybir.ActivationFunctionType.Exp,
                        scale=INVS, bias=expb[:])
                    return et

                def pv_block(j, c, et, u, dst, cols, rhs16, rhs8,
                             with_r):
                    # one (pair, u) PV contribution into dst[:, cols]
                    fp8 = j > 0
                    last = (c == j)
                    if fp8:
                        lhsT = et[:, :, P * u:P * (u + 1)]
                        if with_r:
                            nc.tensor.matmul(
                                r_ps[j][u][:], lhsT, ones8[:],
                                start=(c == 0), stop=last, perf_mode=DR)
                        nc.tensor.matmul(
                            dst[:], lhsT, rhs8[:, 2 * c:2 * c + 2, cols],
                            start=(c == 0), stop=last, perf_mode=DR)
                    else:
                        # j=0, single pair: tile 1 is all-zero for u=0
                        tp = [0] if u == 0 else [0, 1]
                        for ti, tpar in enumerate(tp):
                            lhsT = et[:, tpar, P * u:P * (u + 1)]
                            st_, sp_ = (ti == 0), (ti == len(tp) - 1)
                            if with_r:
                                nc.tensor.matmul(
                                    r_ps[j][u][:], lhsT, ones16[:],
                                    start=st_, stop=sp_)
                            nc.tensor.matmul(
                                dst[:], lhsT, rhs16[:, tpar, cols],
                                start=st_, stop=sp_)

                def emit_norm_half(j, o_half, cols):
                    # u=0 on DVE, u=1 on ACT: halves run in parallel
                    rrec = rrecs[j]
                    for u in range(2):
                        osb = osbp.tile([P, 512], F32, name="osb")
                        qt = 2 * j + u
                        if u == 0:
                            nc.vector.tensor_scalar_mul(
                                osb[:], o_half[u][:], rrec[:, u:u + 1])
                        else:
                            nc.scalar.activation(
                                osb[:], o_half[u][:],
                                mybir.ActivationFunctionType.Copy,
                                scale=rrec[:, u:u + 1])
                        nc.sync.dma_start(
                            out_d[P * qt:P * (qt + 1), cols], osb[:])

                def emit_pv(j, c, et):
                    if c == 0:
                        o_ps[j] = [opa.tile([P, 512], F32, name=f"oA{u}")
                                   for u in range(2)]
                        r_ps[j] = [rp.tile([P, 1], F32, name=f"r_ps{u}")
                                   for u in range(2)]
                    for u in range(2):
                        pv_block(j, c, et, u, o_ps[j][u], slice(0, 512),
                                 v16, v8, with_r=True)
                    if c == j:
                        # h0 done: normalize+DMA it while h1 accumulates
                        rrec = rrp.tile([P, 2], F32, name="rrec")
                        for u in range(2):
                            nc.vector.reciprocal(rrec[:, u:u + 1],
                                                 r_ps[j][u][:])
                        rrecs[j] = rrec
                        emit_norm_half(j, o_ps[j], slice(0, 512))
                        oB = [opb.tile([P, 512], F32, name=f"oB{u}")
                              for u in range(2)]
                        for cc, et2 in enumerate(chunk_ets[j]):
                            for u in range(2):
                                pv_block(j, cc, et2, u, oB[u],
                                         slice(512, D), v16, v8,
                                         with_r=False)
                        emit_norm_half(j, oB, slice(512, D))
                        del o_ps[j], r_ps[j], chunk_ets[j], rrecs[j]

                passes = [(j, c) for j in range(NQC) for c in range(j + 1)]
                pending = []
                for (j, c) in passes:
                    et = emit_pair_scores(j, c)
                    chunk_ets.setdefault(j, []).append(et)
                    pending.append((j, c, et))
                    if len(pending) > PIPE:
                        emit_pv(*pending.pop(0))
                for args in pending:
                    emit_pv(*args)

    nc.finalize()
    return nc


_NC = None


def _get_nc():
    global _NC
    if _NC is None:
        _NC = build()
    return _NC


def prep_inputs(x, Wq, Wk, Wv):
    """Host-side marshaling: shard batch, transpose + cast, constants."""
    import ml_dtypes
    F8NP = ml_dtypes.float8_e4m3

    WqkT16 = np.ascontiguousarray(
        np.stack([np.asarray(Wq).T * VS, np.asarray(Wk).T]),
        dtype=np.float16)
    Wv16 = np.ascontiguousarray(Wv, dtype=np.float16)
    Wv8 = np.ascontiguousarray(np.asarray(Wv) * VS).astype(F8NP)
    id16 = np.eye(P, dtype=np.float16)
    r = np.arange(P)[:, None]
    col = np.arange(QC)[None, :]
    maskb = np.zeros((P, 2, QC), dtype=np.float16)
    maskb[:, 0, :] = np.where(col >= r, 0.0, -25600.0)
    maskb[:, 1, :] = np.where(col >= r + P, 0.0, -25600.0)
    ones8 = np.full((P, 2, 1), VS).astype(F8NP)

    out = []
    for b in range(B):
        xt = np.ascontiguousarray(np.asarray(x[b]).T)
        out.append({
            "xt8": xt.astype(F8NP),
            "xt16": np.ascontiguousarray(xt[:, 0:TB]).astype(np.float16),
            "WqkT": WqkT16, "Wv16": Wv16, "Wv8": Wv8,
            "id16": id16, "maskb": maskb, "ones8": ones8,
        })
    return out


def run(x, Wq, Wk, Wv, **spmd_kwargs):
    from concourse.bass_utils import run_bass_kernel_spmd

    nc = _get_nc()
    in_maps = prep_inputs(x, Wq, Wk, Wv)
    res = run_bass_kernel_spmd(nc, in_maps, core_ids=list(range(B)),
                               **spmd_kwargs)
    out = np.stack([res.results[b]["out"] for b in range(B)], axis=0)
    return out, res


def kernel(x, Wq, Wk, Wv):
    out, _ = run(x, Wq, Wk, Wv)
    return out


# revision 35
# speedup vs baseline: 1.2031x; 1.0108x over previous
"""Single-head causal self-attention on 8 TRN2 NeuronCores.

Problem (hardcoded): x [8, 2048, 1024] f32, Wq/Wk/Wv [1024, 1024] f32.
  Q = x@Wq; K = x@Wk; V = x@Wv
  A = (Q K^T) / sqrt(1024), causal; P = softmax(A); out = P V   -> [8, 2048, 1024] f32

Sharding: batch-parallel - core b computes batch element b, no collectives.

Mixed-precision fp8/fp16 design (rel-err budget 2e-2, fp16 baseline 5.9e-4):
  - The first q-chunk (rows 0..255) has a concentrated softmax (row q
    averages over <=q+1 keys), which amplifies input quantization noise
    into the output; it runs fully in fp16 (projections + attention).
  - Rows 256.. have diffuse softmax (sqrt(sum P^2) <= ~0.2), so fp8e4
    noise attenuates to <1e-2 of output absmax. V = xWv, S^T, and P@V
    run in fp8e4 with DoubleRow perf mode (2 contraction elements / PE
    cycle). T = x G (G = Wq Wk^T) is STORED fp8; numpy-simulating the
    full quant pipeline showed its COMPUTE precision must be fp16 for
    q-cols < 768 (t8 col q only feeds row q's scores; rows 256..767
    still have concentrated-enough softmax that the extra x8-quant
    noise of fp8-computed T doubles their error) while cols 768+
    tolerate the cheaper fp8 DoubleRow compute at zero error change.
  - WqT is host-scaled x16, so G/T/scores carry a uniform x16 that the
    1/512 exp scale removes; G's sigma=1/32 entries then clear the e4m3
    subnormal floor when T is cast. Wv8 is likewise host-scaled x16,
    cancelled by a 16-valued ones vector in the row-sum matmul.
  - Causal masking: a [128,128] identity matmul adds a -1600/-25600 bias
    tile into the diagonal score PSUM; exp(bias*scale) flushes masked
    entries to exact 0 in fp8/fp16. No gpsimd in the attention loop.
  - exp uses bias -3 (E = e^(s-3)) so E stays below e4m3's 240->Inf
    cliff even for 7-sigma scores; the common factor cancels in r.
"""
import numpy as np

import concourse.bacc as bacc
import concourse.bass as bass
import concourse.mybir as mybir
import concourse.tile as tile

F32 = mybir.dt.float32
F16 = mybir.dt.float16
F8 = mybir.dt.float8e4
DR = mybir.MatmulPerfMode.DoubleRow

B = 8
S = 2048
D = 1024
P = 128
ND = D // P          # 8 d-tiles (contraction tiles)
NS = S // P          # 16 s-tiles
QC = 256             # q-chunk for attention
NQC = S // QC        # 8 chunks
INVS = 1.0 / (32.0 * 16.0)  # exp scale: 1/sqrt(d_model) / (x16 of WqT)
EXPB = -3.0               # exp bias: E = e^(s-3), cancels in normalize
VS = 16.0                 # host scale on Wv8, cancelled via ones8=16
TB = 768                  # t8 cols < TB are computed in fp16


def build():
    nc = bacc.Bacc(None, target_bir_lowering=False)

    xt8_d = nc.dram_tensor("xt8", [D, S], F8, kind="ExternalInput")
    xt16_d = nc.dram_tensor("xt16", [D, TB], F16, kind="ExternalInput")
    wqkt_d = nc.dram_tensor("WqkT", [2, D, D], F16, kind="ExternalInput")
    wv16_d = nc.dram_tensor("Wv16", [D, D], F16, kind="ExternalInput")
    wv8_d = nc.dram_tensor("Wv8", [D, D], F8, kind="ExternalInput")
    id_d = nc.dram_tensor("id16", [P, P], F16, kind="ExternalInput")
    # mask bias rows per diagonal tile position (scores carry x16)
    mb_d = nc.dram_tensor("maskb", [P, 2, QC], F16, kind="ExternalInput")
    ones8_d = nc.dram_tensor("ones8", [P, 2, 1], F8, kind="ExternalInput")
    out_d = nc.dram_tensor("out", [S, D], F32, kind="ExternalOutput")

    with tile.TileContext(nc) as tc:
        with (
            tc.tile_pool(name="consts", bufs=1) as consts,
            tc.tile_pool(name="big", bufs=1) as big,
        ):
            ones16 = consts.tile([P, 1], F16)
            nc.gpsimd.memset(ones16[:], 1.0)
            expb = consts.tile([P, 1], F32)
            nc.gpsimd.memset(expb[:], EXPB)

            xt16 = big.tile([P, ND, TB], F16)   # x^T cols 0:TB, fp16
            x8 = big.tile([P, ND, S], F8)       # x^T full, fp8
            tt16 = big.tile([P, ND, QC], F16)   # 16*T^T q-cols 0:256, fp16
            t8 = big.tile([P, ND, S], F8)       # 16*T^T, q-cols 256:, fp8
            v16 = big.tile([P, 2, D], F16)      # V rows 0:256, fp16
            v8 = big.tile([P, NS, D], F8)       # 16*V, fp8
            g16 = big.tile([P, ND, D], F16)     # 16*G, G = Wq Wk^T
            g8 = big.tile([P, ND, D], F8)       # 16*G, fp8
            wv16s = big.tile([P, ND, D], F16)
            wv8s = big.tile([P, ND, D], F8)
            id16 = consts.tile([P, P], F16)
            maskb = consts.tile([P, 2, QC], F16)
            ones8 = consts.tile([P, 2, 1], F8)
            wqk_t = [big.tile([P, 2, ND, 256], F16, name=f"wqk{c}")
                     for c in range(4)]

            # ---- input DMAs, one ring, FIFO in consumption order ----
            wqk_src = wqkt_d[:, :, :].rearrange("w (a p) n -> p w a n", p=P)
            for c in range(4):
                sl = slice(256 * c, 256 * (c + 1))
                nc.sync.dma_start(wqk_t[c][:], wqk_src[:, :, :, sl])
            nc.sync.dma_start(id16[:], id_d[:, :])
            nc.sync.dma_start(maskb[:], mb_d[:, :, :])
            nc.sync.dma_start(ones8[:], ones8_d[:, :, :])
            nc.sync.dma_start(
                xt16[:], xt16_d[:, :].rearrange("(a p) s -> p a s", p=P))
            nc.sync.dma_start(
                x8[:], xt8_d[:, :].rearrange("(a p) s -> p a s", p=P))
            nc.sync.dma_start(
                wv16s[:], wv16_d[:, :].rearrange("(a p) n -> p a n", p=P))
            nc.sync.dma_start(
                wv8s[:], wv8_d[:, :].rearrange("(a p) n -> p a n", p=P))

            with (
                tc.tile_pool(name="projp", bufs=3, space="PSUM") as projp,
                tc.tile_pool(name="warmp", bufs=1, space="PSUM") as warmp,
            ):
                ncopy = 0

                def psum_out(dst_ap, ps, k):
                    if k % 2 == 0:
                        nc.vector.tensor_copy(dst_ap, ps[:])
                    else:
                        nc.scalar.copy(dst_ap, ps[:])

                # PE warmup sized to end as the first W^T chunks land.
                dum = consts.tile([P, 512], F16)
                nc.gpsimd.memset(dum[:], 0.0)
                wtile = warmp.tile([P, 512], F32)
                for _ in range(8):
                    nc.tensor.matmul(wtile[:], dum[:, 0:P], dum[:],
                                     start=True, stop=True)

                # G = Wq Wk^T (fp16): out[d 128 (m), e 256 (c)], group
                # order so level L only needs the first L+1 W^T chunks.
                gpairs = sorted(
                    ((m, c) for m in range(ND) for c in range(D // 256)),
                    key=lambda mc: (max(mc[0] // 2, mc[1]), mc[1], mc[0]))
                for m, c in gpairs:
                    ps = projp.tile([P, 256], F32, name="gps")
                    for a in range(ND):
                        nc.tensor.matmul(
                            ps[:],
                            wqk_t[m // 2][:, 0, a,
                                          P * (m % 2):P * (m % 2 + 1)],
                            wqk_t[c][:, 1, a, :],
                            start=(a == 0), stop=(a == ND - 1))
                    # two copies: fp16 master (DVE) + fp8 (ACT)
                    sl = slice(256 * c, 256 * (c + 1))
                    nc.vector.tensor_copy(g16[:, m, sl], ps[:])
                    nc.scalar.copy(g8[:, m, sl], ps[:])

                # 16*T^T q-cols 0:256 (fp16): out[e 128 (m), q 256]
                for m in range(ND):
                    ps = projp.tile([P, QC], F32)
                    for a in range(ND):
                        nc.tensor.matmul(
                            ps[:],
                            g16[:, a, P * m:P * (m + 1)],
                            xt16[:, a, 0:QC],
                            start=(a == 0), stop=(a == ND - 1))
                    psum_out(tt16[:, m, :], ps, ncopy)
                    ncopy += 1
                # V rows 0:256 (fp16): out[s 128 (i), dv 512 (h)]
                for i in range(2):
                    for h in range(D // 512):
                        ps = projp.tile([P, 512], F32)
                        for a in range(ND):
                            nc.tensor.matmul(
                                ps[:],
                                xt16[:, a, P * i:P * (i + 1)],
                                wv16s[:, a, 512 * h:512 * (h + 1)],
                                start=(a == 0), stop=(a == ND - 1))
                        psum_out(v16[:, i, 512 * h:512 * (h + 1)], ps, ncopy)
                        ncopy += 1

                # 16*T^T q-cols 256:TB: fp16 compute (precision), fp8 store
                for qlo, qw in [(256, 512)]:
                    for m in range(ND):
                        ps = projp.tile([P, 512], F32)
                        for a in range(ND):
                            nc.tensor.matmul(
                                ps[:, 0:qw],
                                g16[:, a, P * m:P * (m + 1)],
                                xt16[:, a, qlo:qlo + qw],
                                start=(a == 0), stop=(a == ND - 1))
                        psum_out(t8[:, m, qlo:qlo + qw], ps[:, 0:qw], ncopy)
                        ncopy += 1
                # 16*T^T q-cols TB:2048: fp8 DoubleRow compute
                for qlo, qw in [(768, 512), (1280, 512), (1792, 256)]:
                    for m in range(ND):
                        ps = projp.tile([P, 512], F32)
                        for a in range(ND // 2):
                            nc.tensor.matmul(
                                ps[:, 0:qw],
                                g8[:, 2 * a:2 * a + 2, P * m:P * (m + 1)],
                                x8[:, 2 * a:2 * a + 2, qlo:qlo + qw],
                                start=(a == 0), stop=(a == ND // 2 - 1),
                                perf_mode=DR)
                        psum_out(t8[:, m, qlo:qlo + qw], ps[:, 0:qw], ncopy)
                        ncopy += 1
                # 16*V (fp8 DoubleRow): out[s 128 (i), dv 512 (h)]
                for i in range(NS):
                    for h in range(D // 512):
                        ps = projp.tile([P, 512], F32)
                        for a in range(ND // 2):
                            nc.tensor.matmul(
                                ps[:],
                                x8[:, 2 * a:2 * a + 2, P * i:P * (i + 1)],
                                wv8s[:, 2 * a:2 * a + 2,
                                     512 * h:512 * (h + 1)],
                                start=(a == 0), stop=(a == ND // 2 - 1),
                                perf_mode=DR)
                        psum_out(v8[:, i, 512 * h:512 * (h + 1)], ps, ncopy)
                        ncopy += 1

            # ---- attention over q-chunks of 256, k-tile PAIRS of 256 ----
            # Pass (j, c) computes S^T + exp for k-tiles 2c, 2c+1 into a
            # paired E tile [P, 2, QC]; PV/r run as deferred PIPE passes
            # (DoubleRow over the k-pair for j>=1, fp16 per-tile for j=0).
            # PV is split into two dv-halves on separate PSUM bank pairs
            # (opa: dv 0:512, opb: dv 512:1024): half-A's normalize+DMA
            # runs while half-B accumulates, so the next chunk's PV never
            # waits on bank recycling (was ~1.2us x 7 chunk boundaries).
            PIPE = 2
            with (
                tc.tile_pool(name="stp", bufs=2, space="PSUM") as stp,
                tc.tile_pool(name="opa", bufs=1, space="PSUM") as opa,
                tc.tile_pool(name="opb", bufs=1, space="PSUM") as opb,
                tc.tile_pool(name="rp", bufs=1, space="PSUM") as rp,
                tc.tile_pool(name="ep", bufs=NQC + PIPE + 2) as ep,
                tc.tile_pool(name="osbp", bufs=4) as osbp,
                tc.tile_pool(name="rrp", bufs=2) as rrp,
            ):
                o_ps = {}
                r_ps = {}
                chunk_ets = {}
                rrecs = {}

                def emit_pair_scores(j, c):
                    # Both k-tiles of the pair accumulate into ONE PSUM
                    # bank as a single group (start only on the very
                    # first matmul: untouched bytes stay pending-zero, so
                    # each half's first write lands on zero), closed by a
                    # single [P, 2, QC] exp -> halved ACT instruction
                    # count and pair-granular st freeing.
                    fp8 = j > 0
                    et = ep.tile([P, 2, QC], F8 if fp8 else F16, name="et")
                    st = stp.tile([P, 2, QC], F32, name="st")
                    n_mm = ND // 2 if fp8 else ND
                    for tpar in range(2):
                        t = 2 * c + tpar
                        tl = t - 2 * j
                        diag = tl >= 0  # diagonal-block k-tile
                        # tl=1 tile: q-cols 0:128 are fully masked; skip
                        # computing them (the id/maskb matmul writes the
                        # -25600 rows onto those pending-zero bytes, so
                        # exp still flushes them to exact 0)
                        qs = slice(P, QC) if tl == 1 else slice(0, QC)
                        for a in range(n_mm):
                            first = (tpar == 0 and a == 0)
                            last = (tpar == 1 and a == n_mm - 1
                                    and not diag)
                            if fp8:
                                nc.tensor.matmul(
                                    st[:, tpar, qs],
                                    x8[:, 2 * a:2 * a + 2, P * t:P * (t + 1)],
                                    t8[:, 2 * a:2 * a + 2,
                                       QC * j + qs.start:QC * j + qs.stop],
                                    start=first, stop=last, perf_mode=DR)
                            else:
                                nc.tensor.matmul(
                                    st[:, tpar, qs],
                                    xt16[:, a, P * t:P * (t + 1)],
                                    tt16[:, a, qs],
                                    start=first, stop=last)
                        if diag:
                            # add -25600 masked-position bias rows
                            nc.tensor.matmul(
                                st[:, tpar, :], id16[:],
                                maskb[:, tl, :],
                                start=False, stop=(tpar == 1))
                    nc.scalar.activation(
                        et[:, :, :], st[:],
                        mybir.ActivationFunctionType.Exp,
                        scale=INVS, bias=expb[:])
                    return et

                def pv_block(j, c, et, u, dst, cols, rhs16, rhs8,
                             with_r):
                    # one (pair, u) PV contribution into dst[:, cols]
                    fp8 = j > 0
                    last = (c == j)
                    if fp8:
                        lhsT = et[:, :, P * u:P * (u + 1)]
                        if with_r:
                            nc.tensor.matmul(
                                r_ps[j][u][:], lhsT, ones8[:],
                                start=(c == 0), stop=last, perf_mode=DR)
                        nc.tensor.matmul(
                            dst[:], lhsT, rhs8[:, 2 * c:2 * c + 2, cols],
                            start=(c == 0), stop=last, perf_mode=DR)
                    else:
                        # j=0, single pair: tile 1 is all-zero for u=0
                        tp = [0] if u == 0 else [0, 1]
                        for ti, tpar in enumerate(tp):
                            lhsT = et[:, tpar, P * u:P * (u + 1)]
                            st_, sp_ = (ti == 0), (ti == len(tp) - 1)
                            if with_r:
                                nc.tensor.matmul(
                                    r_ps[j][u][:], lhsT, ones16[:],
                                    start=st_, stop=sp_)
                            nc.tensor.matmul(
                                dst[:], lhsT, rhs16[:, tpar, cols],
                                start=st_, stop=sp_)

                def emit_norm_half(j, o_half, cols):
                    # u=0 on DVE, u=1 on ACT: halves run in parallel
                    rrec = rrecs[j]
                    for u in range(2):
                        osb = osbp.tile([P, 512], F32, name="osb")
                        qt = 2 * j + u
                        if u == 0:
                            nc.vector.tensor_scalar_mul(
                                osb[:], o_half[u][:], rrec[:, u:u + 1])
                        else:
                            nc.scalar.activation(
                                osb[:], o_half[u][:],
                                mybir.ActivationFunctionType.Copy,
                                scale=rrec[:, u:u + 1])
                        nc.sync.dma_start(
                            out_d[P * qt:P * (qt + 1), cols], osb[:])

                def emit_pv(j, c, et):
                    if c == 0:
                        o_ps[j] = [opa.tile([P, 512], F32, name=f"oA{u}")
                                   for u in range(2)]
                        r_ps[j] = [rp.tile([P, 1], F32, name=f"r_ps{u}")
                                   for u in range(2)]
                    for u in range(2):
                        pv_block(j, c, et, u, o_ps[j][u], slice(0, 512),
                                 v16, v8, with_r=True)
                    if c == j:
                        # h0 done: normalize+DMA it while h1 accumulates
                        rrec = rrp.tile([P, 2], F32, name="rrec")
                        for u in range(2):
                            nc.vector.reciprocal(rrec[:, u:u + 1],
                                                 r_ps[j][u][:])
                        rrecs[j] = rrec
                        emit_norm_half(j, o_ps[j], slice(0, 512))
                        oB = [opb.tile([P, 512], F32, name=f"oB{u}")
                              for u in range(2)]
                        for cc, et2 in enumerate(chunk_ets[j]):
                            for u in range(2):
                                pv_block(j, cc, et2, u, oB[u],
                                         slice(512, D), v16, v8,
                                         with_r=False)
                        emit_norm_half(j, oB, slice(512, D))
                        del o_ps[j], r_ps[j], chunk_ets[j], rrecs[j]

                passes = [(j, c) for j in range(NQC) for c in range(j + 1)]
                pending = []
                for (j, c) in passes:
                    et = emit_pair_scores(j, c)
                    chunk_ets.setdefault(j, []).append(et)
                    pending.append((j, c, et))
                    if len(pending) > PIPE:
                        emit_pv(*pending.pop(0))
                for args in pending:
                    emit_pv(*args)

    nc.finalize()
    return nc


_NC = None


def _get_nc():
    global _NC
    if _NC is None:
        _NC = build()
    return _NC


def prep_inputs(x, Wq, Wk, Wv):
    """Host-side marshaling: shard batch, transpose + cast, constants."""
    import ml_dtypes
    F8NP = ml_dtypes.float8_e4m3

    WqkT16 = np.ascontiguousarray(
        np.stack([np.asarray(Wq).T * VS, np.asarray(Wk).T]),
        dtype=np.float16)
    Wv16 = np.ascontiguousarray(Wv, dtype=np.float16)
    Wv8 = np.ascontiguousarray(np.asarray(Wv) * VS).astype(F8NP)
    id16 = np.eye(P, dtype=np.float16)
    r = np.arange(P)[:, None]
    col = np.arange(QC)[None, :]
    maskb = np.zeros((P, 2, QC), dtype=np.float16)
    maskb[:, 0, :] = np.where(col >= r, 0.0, -25600.0)
    maskb[:, 1, :] = np.where(col >= r + P, 0.0, -25600.0)
    ones8 = np.full((P, 2, 1), VS).astype(F8NP)

    out = []
    for b in range(B):
        xt = np.ascontiguousarray(np.asarray(x[b]).T)
        out.append({
            "xt8": xt.astype(F8NP),
            "xt16": np.ascontiguousarray(xt[:, 0:TB]).astype(np.float16),
            "WqkT": WqkT16, "Wv16": Wv16, "Wv8": Wv8,
            "id16": id16, "maskb": maskb, "ones8": ones8,
        })
    return out


def run(x, Wq, Wk, Wv, **spmd_kwargs):
    from concourse.bass_utils import run_bass_kernel_spmd

    nc = _get_nc()
    in_maps = prep_inputs(x, Wq, Wk, Wv)
    res = run_bass_kernel_spmd(nc, in_maps, core_ids=list(range(B)),
                               **spmd_kwargs)
    out = np.stack([res.results[b]["out"] for b in range(B)], axis=0)
    return out, res


def kernel(x, Wq, Wk, Wv):
    out, _ = run(x, Wq, Wk, Wv)
    return out
